# revision 32
# baseline (speedup 1.0000x reference)
"""Trainium2 Bass kernel for nn_CFAConv (cross-feature attention + conv block).

Self-contained: takes full unsharded inputs, shards (batch, image-half) across
8 NeuronCores, runs one SPMD Bass/Tile NEFF, and combines partial results on
the host.

Math (validated against the jax reference in numpy):
  x1 = w_q@in0 + b_q ; x2 = w_k@in0 + b_k ; x3 = w_v@in1 + b_v  (1x1 convs)
  aff = softmax_j(x2^T x3) ; x0 = x1 @ aff
  x0' = gelu(bn0(w_o@x0 + b_o))
  y = gelu(bn(conv3x3(concat(x0', in0)))) ; y = gelu(bn(conv3x3(y)))
  out = max_spatial(y + x0')
On-device simplifications:
  - softmax over j is invariant to per-column shifts => b_k drops entirely
  - x2^T(x3 + b_v) = x2^T x3 + (x2^T b_v) 1^T    => fold b_v into x3
  - (x1 + b_q 1^T) @ aff = x1@aff + b_q 1^T (aff columns sum to 1)
    => fold w_o@b_q into the out-projection bias (host-side)
  - eval-mode BN folds to per-channel scale/bias, fused into the gelu ACT op
  - softmax normalization deferred past the x1@exp(S) matmul (divide x0 by
    column sums); sums via a 5-level bf16 DVE pre-sum tree + one ones-matmul
  - no max-subtraction in softmax: |S| <= ~60 here; exp fits fp32 (max ~e88)
Precision: bf16 operands with fp32 PSUM accumulation for the attention path;
the two 3x3 convs run in fp8e4m3 with DoubleRow perf mode (2 contraction
tiles per pass at 0.5 cycles/row):
  - conv0 x0'-half: weights + acts naive fp8 (x0' is small vs in0 => cheap)
  - conv0 in0-half: weights hi+lo fp8 split, in0 hi+lo fp8 split (host-side),
    3-term product (Wh Xh + Wh Xl + Wl Xh)
  - conv1: weights hi+lo (host), c0 hi+lo split on DVE, 3-term
  (numpy bit-model: 1.3e-2 final rel err vs the 2e-2 budget)
Scheduling (v2): DMAs are packed (fewer issues; the shared HWDGE serializes
at ~630ns per DMA) and ordered so the projection inputs stream in consumption
order; x2/x1t/x3 live in per-chunk tiles so each S matmul gates only on its
own quarter; the attention is software-pipelined: S+exp of k-tile kt+1 are
interleaved into the x0 h-blocks of k-tile kt, the colsum tree runs per
h-block, and the out-projection is folded into the same loop. conv0's
in0-half partial sums (27 DoubleRow passes each) fill the PE slack of the
ACT-exp-bound attention phase.
Sharding: 8 cores = (4 batches) x (top/bottom image half). Each core computes
a 34-row window (32 owned + halo) so the two 3x3 convs need no communication;
per-row maxes [256, 34] go to the host which slices owned rows and reduces.
"""

from contextlib import ExitStack

import ml_dtypes
import numpy as np

import concourse.bass as bass
import concourse.tile as tile
from concourse import bacc, mybir
from concourse.bass_utils import run_bass_kernel_spmd

B, C, H, W = 4, 256, 64, 64
Ch = C // 2          # 128
N = H * W            # 4096
ROWS = 34            # per-core row window (32 owned + 2 halo)
KW = ROWS * W        # 2176 window positions
EPS = 1e-5

F32 = mybir.dt.float32
BF16 = mybir.dt.bfloat16
F8 = mybir.dt.float8e4
AF = mybir.ActivationFunctionType
AX = mybir.AxisListType
DR = mybir.MatmulPerfMode.DoubleRow
BF16NP = ml_dtypes.bfloat16
F8NP = ml_dtypes.float8_e4m3

# attention k-tiles over the 2176-column window
K_TILES = [(0, 512), (512, 512), (1024, 512), (1536, 512), (2048, 128)]
# conv output row-tiles (local rows 1..34 of the 36-row padded buffer)
ROW_TILES = [(1, 8), (9, 8), (17, 8), (25, 8), (33, 2)]

_CACHED = {}


def build_program():
    nc = bacc.Bacc("TRN2", target_bir_lowering=False, debug=False)

    def din(name, shape, dt=F32):
        return nc.dram_tensor(name, shape, dt, kind="ExternalInput").ap()

    # in0/in1 ship as fp8 hi|lo pairs packed per chunk (same bytes as bf16)
    in0p_d = din("in0p", [128, 2, 2 * N], F8)
    in1p_d = din("in1p", [128, 2, 2 * KW], F8)
    # in0 conv window, fp8 hi/lo, pre-padded to 66 cols (zero side columns)
    in0h_d = din("in0h", [C, ROWS * 66], F8)
    in0l_d = din("in0l", [C, ROWS * 66], F8)
    # packed small weights: one DMA each (HWDGE issue is ~630ns/DMA)
    wqk_d = din("wqk", [128, 8, 128], F8)      # [wq_h|wq_l|wk_h|wk_l] (2cc each)
    wvhl_d = din("wvhl", [128, 4, 128], F8)    # [wv_h|wv_l]
    wvoi_d = din("wvoi", [128, 384], BF16)     # wo(256) | id(128)
    biasb_d = din("biasb", [128, 13])          # bias6 (12) | bv (1)
    # conv weights: [x0-half naive, in0 hi, in0 lo, w1 hi, w1 lo] (tap, ci, o)
    w01_d = din("w01", [128, 90, C], F8)
    out = nc.dram_tensor("out", [C, ROWS], F32, kind="ExternalOutput").ap()
    outv = out.rearrange("(a p) r -> p a r", a=2)

    with tile.TileContext(nc) as tc, ExitStack() as ctx:
        persist = ctx.enter_context(tc.tile_pool(name="persist", bufs=1))
        psum = ctx.enter_context(tc.tile_pool(name="psum", bufs=2, space="PSUM"))
        psum1 = ctx.enter_context(tc.tile_pool(name="psum1", bufs=2, space="PSUM"))
        small = ctx.enter_context(tc.tile_pool(name="small", bufs=3))
        small4 = ctx.enter_context(tc.tile_pool(name="small4", bufs=4))

        # ---- input DMAs, in consumption order (transfers serialize at
        # ~358GB/s; HWDGE issues at ~630ns) ----
        # Queue strategy: no input DMAs on the scalar (ACT) queue — each
        # dispatch blocks the ACT sequencer ~1.26us, delaying the first exps.
        # Front-critical inputs go on the SP/HWDGE queue in consumption
        # order (in1a early: it gates S(0,0) and so the whole ACT exp chain).

        wqk_s = persist.tile([128, 8, 128], F8, tag="wqk")
        nc.sync.dma_start(out=wqk_s, in_=wqk_d)
        wqh_s, wql_s = wqk_s[:, 0:2, :], wqk_s[:, 2:4, :]
        wkh_s, wkl_s = wqk_s[:, 4:6, :], wqk_s[:, 6:8, :]
        in0q = [persist.tile([128, 2, 1024], F8, tag=f"in0q{q}",
                             name=f"in0q{q}") for q in range(8)]
        nc.sync.dma_start(out=in0q[0], in_=in0p_d[:, :, 0:1024])
        in1q = [persist.tile([128, 2, 1024], F8, tag="in1a1", name="in1a1"),
                persist.tile([128, 2, 1024], F8, tag="in1a2", name="in1a2"),
                persist.tile([128, 2, 2304], F8, tag="in1b", name="in1b")]
        nc.sync.dma_start(out=in1q[0], in_=in1p_d[:, :, 0:1024])
        wvhl_s = persist.tile([128, 4, 128], F8, tag="wvhl")
        nc.sync.dma_start(out=wvhl_s, in_=wvhl_d)
        wvh_s, wvl_s = wvhl_s[:, 0:2, :], wvhl_s[:, 2:4, :]
        wvoi_s = persist.tile([128, 384], BF16, tag="wvoi")
        nc.sync.dma_start(out=wvoi_s, in_=wvoi_d)
        wo_s = wvoi_s[:, 0:256]
        id_s = wvoi_s[:, 256:384]
        biasb_s = persist.tile([128, 13], F32, tag="biasb")
        nc.sync.dma_start(out=biasb_s, in_=biasb_d)
        ao_s, bo_s = biasb_s[:, 0:2], biasb_s[:, 2:4]
        a0_s, b0_s = biasb_s[:, 4:6], biasb_s[:, 6:8]
        a1_s, b1_s = biasb_s[:, 8:10], biasb_s[:, 10:12]
        bv_s = biasb_s[:, 12:13]
        nc.sync.dma_start(out=in0q[1], in_=in0p_d[:, :, 1024:2048])
        nc.sync.dma_start(out=in1q[1], in_=in1p_d[:, :, 1024:2048])
        for q in range(2, 8):
            nc.sync.dma_start(out=in0q[q],
                              in_=in0p_d[:, :, q * 1024:(q + 1) * 1024])

        # conv inputs/weights (needed from ~28us; back of the SP queue)
        convbuf = ctx.enter_context(tc.tile_pool(name="convbuf", bufs=1))
        in0h_s = convbuf.tile([128, 2, 36, 66], F8, tag="in0h")
        in0l_s = convbuf.tile([128, 2, 36, 66], F8, tag="in0l")
        nc.sync.dma_start(out=in1q[2], in_=in1p_d[:, :, 2048:2 * KW])
        nc.sync.dma_start(
            out=in0h_s[:, :, 1:35, :],
            in_=in0h_d.rearrange("(a p) n -> p a n", a=2))
        nc.sync.dma_start(
            out=in0l_s[:, :, 1:35, :],
            in_=in0l_d.rearrange("(a p) n -> p a n", a=2))
        w01_s = persist.tile([128, 90, C], F8, tag="w01")
        nc.sync.dma_start(out=w01_s, in_=w01_d)
        w0x_s = w01_s[:, 0:18]
        w0inh_s = w01_s[:, 18:36]
        w0inl_s = w01_s[:, 36:54]
        w1h_s = w01_s[:, 54:72]
        w1l_s = w01_s[:, 72:90]

        ones_s = persist.tile([128, 1], BF16, tag="ones")
        nc.vector.memset(ones_s, 1.0)
        ones_row = persist.tile([1, 128], BF16, tag="ones_row")
        nc.vector.memset(ones_row, 1.0)

        # p-state warmup: the PE ramps to full clock only after 3us of
        # continuous execution; spin it on throwaway matmuls while the first
        # input DMAs are in flight
        spin_src = persist.tile([128, 256], BF16, tag="spin_src")
        nc.vector.memset(spin_src, 1.0)
        spin_ps = psum1.tile([128, 512], F32, tag="ps_cv", name="spin_ps")
        for _ in range(5):
            nc.tensor.matmul(spin_ps[0:1, :256], ones_s, spin_src,
                             start=True, stop=True)
        spin_sink = small.tile([1, 16], F32, tag="spin_sink")
        nc.vector.tensor_copy(spin_sink, spin_ps[0:1, :16])  # ring consumer

        # ---- conv pad zeroing on the otherwise-idle Pool engine ----
        TOPR, BOTR0 = 19, 16  # top rows [0,19), bottom rows [16,36)
        # ybuf in THREE pieces so conv0's row tiles gate on the narrowest
        # set of out-projection writers (tile deps are whole-tile):
        # T rows [0,19), M rows [16,27), B rows [24,36)
        ybufT = convbuf.tile([128, 2, TOPR, 66], F8, tag="ybufT")
        ybufM = convbuf.tile([128, 2, 11, 66], F8, tag="ybufM")
        ybufB = convbuf.tile([128, 2, 12, 66], F8, tag="ybufB")
        YPIECES = [("T", ybufT, 0, 19), ("M", ybufM, 16, 27),
                   ("B", ybufB, 24, 36)]
        c0hT = convbuf.tile([128, 2, TOPR, 66], F8, tag="c0hT")
        c0hB = convbuf.tile([128, 2, 36 - BOTR0, 66], F8, tag="c0hB")
        c0lT = convbuf.tile([128, 2, TOPR, 66], F8, tag="c0lT")
        c0lB = convbuf.tile([128, 2, 36 - BOTR0, 66], F8, tag="c0lB")
        c0f = convbuf.tile([128, 2, ROWS, W], BF16, tag="c0f")
        def pad_memsets():
            # on DVE (mid-queue): Pool must stay free for the x3 bias-adds,
            # and the ACT/SP queues are on critical paths
            for tl in (in0h_s, in0l_s):
                nc.vector.memset(tl[:, :, 0, :], 0.0)
                nc.vector.memset(tl[:, :, 35, :], 0.0)
            for tl in (ybufT, c0hT, c0lT):
                nc.vector.memset(tl[:, :, 0, :], 0.0)
                nc.vector.memset(tl[:, :, 1:, 0:1], 0.0)
                nc.vector.memset(tl[:, :, 1:, 65:66], 0.0)
            nc.vector.memset(ybufM[:, :, :, 0:1], 0.0)
            nc.vector.memset(ybufM[:, :, :, 65:66], 0.0)
            nc.vector.memset(ybufB[:, :, 11, :], 0.0)
            nc.vector.memset(ybufB[:, :, :11, 0:1], 0.0)
            nc.vector.memset(ybufB[:, :, :11, 65:66], 0.0)
            for tl in (c0hB, c0lB):
                nc.vector.memset(tl[:, :, 36 - BOTR0 - 1, :], 0.0)
                nc.vector.memset(tl[:, :, :36 - BOTR0 - 1, 0:1], 0.0)
                nc.vector.memset(tl[:, :, :36 - BOTR0 - 1, 65:66], 0.0)

        # ---- projections into per-chunk tiles (deps gate per quarter) ----
        x2_t = [persist.tile([128, 512], BF16, tag=f"x2_{q}", name=f"x2_{q}")
                for q in range(8)]
        x1t_t = [persist.tile([128, 4, Ch], BF16, tag=f"x1t_{q}",
                              name=f"x1t_{q}") for q in range(8)]
        x3_t = [persist.tile([128, K_TILES[kt][1]], BF16, tag=f"x3_{kt}",
                             name=f"x3_{kt}") for kt in range(5)]

        def proj_x2(jc):
            # x2 chunk: 3-term fp8 DoubleRow (Wh Xh + Wl Xh + Wh Xl), one
            # accumulation group in one PSUM bank. On the ps_acc ring
            # (DVE-drained): the ps_S ring is reserved for x3 + S pairs
            # whose consumers (ACT exps) are slow.
            ps2 = psum1.tile([128, 512], F32, tag="ps_acc",
                             name=f"ps2_{jc}")
            t = in0q[jc]
            terms = [(wkh_s, 0), (wkl_s, 0), (wkh_s, 512)]
            for i, (w_s, xo) in enumerate(terms):
                nc.tensor.matmul(ps2, w_s, t[:, :, xo:xo + 512],
                                 start=(i == 0), stop=(i == 2),
                                 perf_mode=DR)
            nc.vector.tensor_copy(x2_t[jc], ps2)

        def proj_x1t(jc, tag="ps_acc"):
            # x1t naive fp8 (Wh Xh only): x1's error enters x0 linearly, not
            # through the exp; numpy bit-model 1.49e-2 vs the 2e-2 budget
            ps1 = psum1.tile([128, 512], F32, tag=tag, name=f"ps1_{jc}")
            t = in0q[jc]
            for js in range(4):
                nc.tensor.matmul(
                    ps1[:, js * 128:js * 128 + Ch],
                    t[:, :, js * 128:js * 128 + 128], wqh_s,
                    start=(js == 0), stop=(js == 3),
                    perf_mode=DR)
            nc.vector.tensor_copy(x1t_t[jc],
                                  ps1.rearrange("p (a c) -> p a c", c=Ch))

        def x3_tile(kt):
            k0, ksz = K_TILES[kt]
            iq, off, hw = ((kt, 0, 512) if k0 < 1024 else
                           (2, k0 - 1024, 1152))
            ps3 = psum.tile([128, 2, 512], F32, tag="ps_S", name=f"ps3_{kt}")
            t = in1q[iq]
            terms = [(wvh_s, 0), (wvl_s, 0), (wvh_s, hw)]
            for i, (w_s, xo) in enumerate(terms):
                nc.tensor.matmul(ps3[:, 0, :ksz], w_s,
                                 t[:, :, xo + off:xo + off + ksz],
                                 start=(i == 0), stop=(i == 2),
                                 perf_mode=DR)
            # x3 = psum + b_v : folds the v-bias into the affinity logits
            nc.vector.tensor_scalar_add(x3_t[kt][:, :ksz], ps3[:, 0, :ksz],
                                        bv_s)

        # ---- conv0 in0-half partials (fill PE slack in the attention) ----
        inpart = [persist.tile([128, 512], BF16, tag=f"inpart{g}",
                               name=f"inpart{g}") for g in range(10)]
        terms_in0 = [(w0inh_s, in0h_s), (w0inh_s, in0l_s), (w0inl_s, in0h_s)]
        part_ps = {}

        def partial_chunk(g, p0, p1):
            """DoubleRow passes [p0,p1) of the 27 for conv0's in0-half group
            g=(rt,oc); chunked so the passes can be spread between attention
            h-blocks as PE filler; evicted to SBUF bf16 on the last chunk."""
            (r0, nr), oc = ROW_TILES[g // 2], g % 2
            if p0 == 0:
                part_ps[g] = psum1.tile([128, 512], F32, tag="ps_cv",
                                        name=f"cv{g}")
            ps = part_ps[g]
            pcv = ps[:, :nr * W].rearrange("p (r w) -> p r w", w=W)
            for i_mm in range(p0, p1):
                w_s, x_s = terms_in0[i_mm // 9]
                dh, dw = divmod(i_mm % 9, 3)
                nc.tensor.matmul(
                    pcv,
                    w_s[:, (i_mm % 9) * 2:(i_mm % 9) * 2 + 2,
                        oc * 128:(oc + 1) * 128],
                    x_s[:, :, r0 + dh - 1:r0 + dh - 1 + nr, dw:dw + W],
                    start=(i_mm == 0), stop=(i_mm == 26),
                    perf_mode=DR)
            if p1 == 27:
                nc.vector.tensor_copy(inpart[g][:, :nr * W], ps[:, :nr * W])

        def emit_in0_partial(g):
            partial_chunk(g, 0, 27)

        # ---- software-pipelined attention ----
        attn = ctx.enter_context(tc.tile_pool(name="attn", bufs=4))
        attn2 = ctx.enter_context(tc.tile_pool(name="attn2", bufs=2))
        attnE = ctx.enter_context(tc.tile_pool(name="attnE", bufs=18))
        x0n_t = [persist.tile([128, 512], BF16, tag=f"x0n{kt}",
                              name=f"x0n{kt}") for kt in range(len(K_TILES))]
        poS_t = [persist.tile([128, 2, 512], BF16, tag=f"poS{kt}",
                              name=f"poS{kt}") for kt in range(len(K_TILES))]
        expS_cur = {}

        def S_block(kt, mh):
            """S matmuls for chunk-pair mh of k-tile kt + the exp into a
            per-pair expS ring tile (x0 then gates on ONE exp, not four)."""
            k0, ksz = K_TILES[kt]
            et = attnE.tile([128, 2, 512], BF16, tag="expS",
                            name=f"expS{kt}_{mh}")
            expS_cur[(kt, mh)] = et
            sp = psum.tile([128, 2, 512], F32, tag="ps_S",
                           name=f"S{kt}_{mh}")
            for i in range(2):
                m = 2 * mh + i
                nc.tensor.matmul(
                    sp[:, i, :ksz],
                    x2_t[m // 4][:, (m % 4) * 128:(m % 4 + 1) * 128],
                    x3_t[kt][:, :ksz],
                    start=True, stop=True)
            nc.scalar.activation(et[:, :, :ksz], sp[:, :, :ksz], AF.Exp)

        part_x1 = {}

        def proj_x1t_chunk(jc, c0, c1):
            """passes [c0,c1) of quarter jc's 4-pass x1t group (deferred
            into the kt0 pair-loop as PE filler; psum on the ps_cv ring)"""
            if c0 == 0:
                part_x1[jc] = psum1.tile([128, 512], F32, tag="ps_cv",
                                         name=f"ps1_{jc}")
            ps1 = part_x1[jc]
            t = in0q[jc]
            for js in range(c0, c1):
                nc.tensor.matmul(
                    ps1[:, js * 128:js * 128 + Ch],
                    t[:, :, js * 128:js * 128 + 128], wqh_s,
                    start=(js == 0), stop=(js == 3),
                    perf_mode=DR)
            if c1 == 4:
                nc.vector.tensor_copy(x1t_t[jc],
                                      ps1.rearrange("p (a c) -> p a c", c=Ch))

        # ---- front: x2 streams behind the input DMAs; x1t for quarters 2-7
        # is deferred into the kt0 pair-loop as PE filler (the attention
        # inner loop is ACT-exp-bound); S(kt0) starts once q0/q1 + x3t0 up ----
        proj_x2(0)
        proj_x1t(0)
        x3_tile(0)
        proj_x2(1)
        proj_x1t(1)
        S_block(0, 0)
        S_block(0, 1)
        S_block(0, 2)
        S_block(0, 3)
        x3_tile(1)
        proj_x2(2)
        S_block(0, 4)
        S_block(0, 5)
        proj_x2(3)
        S_block(0, 6)
        S_block(0, 7)
        proj_x2(4)
        S_block(0, 8)
        S_block(0, 9)
        proj_x2(5)
        S_block(0, 10)
        S_block(0, 11)
        proj_x2(6)
        S_block(0, 12)
        S_block(0, 13)
        proj_x2(7)
        S_block(0, 14)
        S_block(0, 15)
        pad_memsets()

        for kt, (k0, ksz) in enumerate(K_TILES):
            octs = attn2.tile([128, 4, 512], BF16, tag="oct",
                              name=f"oct{kt}")
            x0p = psum1.tile([128, 512], F32, tag="ps_acc", name=f"x0p{kt}")
            pr_t, qr_t = {}, {}
            filled = 0
            for mh in range(16):
                if kt + 1 < len(K_TILES):
                    S_block(kt + 1, mh)
                # PE filler BEFORE the exp-gated x0 pair, so PE chews on it
                # while ACT finishes exp(kt, mh)
                if kt == 0:
                    if 1 <= mh <= 6:
                        q = 2 + (mh - 1)
                        proj_x1t_chunk(q, 0, 2)
                        proj_x1t_chunk(q, 2, 4)
                    elif 7 <= mh <= 9:
                        x3_tile(mh - 5)   # x3 tiles 2, 3, 4
                    elif mh >= 12:
                        # group g0 fills kt0's tail (w01 lands ~22us)
                        tgt = 27 * (mh - 11) // 4
                        partial_chunk(0, filled, tgt)
                        filled = tgt
                else:
                    # two conv0-in0 partial groups per kt (54 DR passes)
                    tgt = 54 * (mh + 1) // 16
                    while filled < tgt:
                        g = 2 * kt - 1 + filled // 27
                        p = filled % 27
                        e = min(27, p + tgt - filled)
                        partial_chunk(g, p, e)
                        filled += e - p
                et = expS_cur[(kt, mh)]
                for i in range(2):
                    m = 2 * mh + i
                    nc.tensor.matmul(x0p[:, :ksz], x1t_t[m // 4][:, m % 4, :],
                                     et[:, i, :ksz],
                                     start=(m == 0), stop=(m == 31))
                pr = attn.tile([128, 512], BF16, tag="pr",
                               name=f"pr{kt}_{mh}")
                pr_t[mh] = pr
                nc.vector.tensor_add(pr[:, :ksz], et[:, 0, :ksz],
                                     et[:, 1, :ksz])
                if mh % 2 == 1:
                    qr = attn.tile([128, 512], BF16, tag="qr",
                                   name=f"qr{kt}_{mh//2}")
                    qr_t[mh // 2] = qr
                    nc.vector.tensor_add(qr[:, :ksz], pr_t[mh - 1][:, :ksz],
                                         pr_t[mh][:, :ksz])
                if mh % 4 == 3:
                    nc.vector.tensor_add(octs[:, mh // 4, :ksz],
                                         qr_t[mh // 2 - 1][:, :ksz],
                                         qr_t[mh // 2][:, :ksz])
            hexs = attn2.tile([128, 2, 512], BF16, tag="hex", name=f"hex{kt}")
            for i in range(2):
                nc.vector.tensor_add(hexs[:, i, :ksz], octs[:, 2 * i, :ksz],
                                     octs[:, 2 * i + 1, :ksz])
            top = attn2.tile([128, 512], BF16, tag="top", name=f"top{kt}")
            nc.vector.tensor_add(top[:, :ksz], hexs[:, 0, :ksz],
                                 hexs[:, 1, :ksz])
            ssum_t = psum1.tile([128, 512], F32, tag="ps_cv", name=f"ssum{kt}")
            nc.tensor.matmul(ssum_t[0:1, :ksz], ones_s, top[:, :ksz],
                             start=True, stop=True)
            sinv = small.tile([1, 512], BF16, tag="sinv")
            with nc.allow_low_precision(
                    reason="bf16 1/colsum only scales the softmax "
                           "normalization; 1.2e-2 in the numpy bit model"):
                nc.vector.reciprocal(sinv[:, :ksz], ssum_t[0:1, :ksz])
            # broadcast 1/colsum to all partitions with a 1-row bf16 matmul
            bcast = psum1.tile([128, 512], F32, tag="ps_cv",
                               name=f"sinv_bcast{kt}")
            nc.tensor.matmul(bcast[:, :ksz], ones_row, sinv[:, :ksz],
                             start=True, stop=True)
            sinvb = small.tile([128, 512], F32, tag="sinvb")
            nc.vector.tensor_copy(sinvb[:, :ksz], bcast[:, :ksz])
            nc.vector.tensor_mul(x0n_t[kt][:, :ksz], x0p[:, :ksz],
                                 sinvb[:, :ksz])
            # out-projection matmuls here; psum evicted to SBUF f32 so the
            # gelus (batched post-attention to avoid exp<->gelu activation
            # table thrash) don't pin PSUM banks across the attention
            for oc in range(2):
                po = psum1.tile([128, 512], F32, name=f"po{kt}_{oc}",
                                tag="ps_acc" if oc == 0 else "ps_cv")
                nc.tensor.matmul(po[:, :ksz],
                                 wo_s[:, oc * 128:(oc + 1) * 128],
                                 x0n_t[kt][:, :ksz],
                                 start=True, stop=True)
                nc.vector.tensor_copy(poS_t[kt][:, oc, :ksz], po[:, :ksz])

        # ---- out-projection + bn0 + gelu -> x0' (fp8) into ybuf. Kept as a
        # separate loop so the ACT stream stays [exps..., gelus...] — mixing
        # them costs a 1283ns activation-table load per switch. ----
        def outproj(kt):
            k0, ksz = K_TILES[kt]
            nr = ksz // W
            r0 = 1 + kt * 8
            for oc in range(2):
                po = psum1.tile([128, 512], F32, name=f"po{kt}_{oc}",
                                tag="ps_acc" if oc == 0 else "ps_cv")
                nc.tensor.matmul(po[:, :ksz],
                                 wo_s[:, oc * 128:(oc + 1) * 128],
                                 x0n_t[kt][:, :ksz],
                                 start=True, stop=True)
                pv = po[:, :ksz].rearrange("p (r w) -> p r w", w=W)
                for _, tl, p0, p1 in YPIECES:
                    lo, hi = max(r0, p0), min(r0 + nr, p1)
                    if lo < hi:
                        nc.scalar.activation(
                            tl[:, oc, lo - p0:hi - p0, 1:65],
                            pv[:, lo - r0:hi - r0],
                            AF.Gelu, bias=bo_s[:, oc:oc + 1],
                            scale=ao_s[:, oc:oc + 1])

        def rd(r0):
            """conv read window [r0-1, r0-1+nr+2) maps to exactly one tile"""
            return ("T", r0) if r0 < 16 else ("B", r0 - BOTR0)

        def yrd(r0, nr):
            """ybuf piece whose range contains rows [r0-1, r0+nr+1)"""
            for _, tl, p0, p1 in YPIECES:
                if r0 - 1 >= p0 and r0 + nr + 1 <= p1:
                    return tl, r0 - p0
            raise AssertionError(r0)

        def split_rows2(r0, nrows):
            parts = []
            t_hi = min(r0 + nrows, TOPR)
            if r0 < TOPR:
                parts.append(("T", r0, t_hi - r0, 0))
            b_lo = max(r0, BOTR0)
            if r0 + nrows > BOTR0:
                parts.append(("B", b_lo - BOTR0, r0 + nrows - b_lo, b_lo - r0))
            return parts

        # ---- conv0: x0'-half naive fp8 DoubleRow on top of the re-injected
        # in0-half partial (identity matmul opens the accumulation).
        # Tiles 0/1 read only ybufT (complete before outproj kt4), so they
        # are emitted first to cover kt4's serial ssum->recip->bcast->x0n
        # chain; outproj(kt4) then unblocks the ybufB tiles. ----
        def conv0_tile(ri):
            r0, nr = ROW_TILES[ri]
            for oc in range(2):
                pc = psum1.tile([128, 512], F32, name=f"c0ps{ri}_{oc}",
                                tag="ps_acc" if oc == 0 else "ps_cv")
                pcv = pc[:, :nr * W].rearrange("p (r w) -> p r w", w=W)
                ysrc, lr0 = yrd(r0, nr)
                nc.tensor.matmul(pc[:, :nr * W], id_s,
                                 inpart[ri * 2 + oc][:, :nr * W],
                                 start=True, stop=False)
                for t9 in range(9):
                    dh, dw = divmod(t9, 3)
                    nc.tensor.matmul(
                        pcv,
                        w0x_s[:, t9 * 2:t9 * 2 + 2, oc * 128:(oc + 1) * 128],
                        ysrc[:, :, lr0 + dh - 1:lr0 + dh - 1 + nr, dw:dw + W],
                        start=False, stop=(t9 == 8),
                        perf_mode=DR)
                nc.scalar.activation(
                    c0f[:, oc, r0 - 1:r0 - 1 + nr, :], pcv,
                    AF.Gelu, bias=b0_s[:, oc:oc + 1], scale=a0_s[:, oc:oc + 1])
                # hi/lo split of c0 for conv1's 3-term product (DVE), into
                # the top/bottom tiles (boundary rows land in both)
                for w2, lr2, n2, src2 in split_rows2(r0, nr):
                    chh = c0hT if w2 == "T" else c0hB
                    cll = c0lT if w2 == "T" else c0lB
                    s2 = r0 - 1 + src2
                    nc.vector.tensor_copy(chh[:, oc, lr2:lr2 + n2, 1:65],
                                          c0f[:, oc, s2:s2 + n2, :])
                    nc.vector.tensor_sub(cll[:, oc, lr2:lr2 + n2, 1:65],
                                         c0f[:, oc, s2:s2 + n2, :],
                                         chh[:, oc, lr2:lr2 + n2, 1:65])

        for kt in range(4):
            outproj(kt)
            if kt == 1:
                emit_in0_partial(9)
        conv0_tile(0)
        conv0_tile(1)
        conv0_tile(2)
        outproj(4)
        conv0_tile(3)
        conv0_tile(4)

        # ---- conv1: 256 -> 256, 3-term DoubleRow fp8, bn + gelu,
        #      + x0' residual, row-max; one output DMA per row-tile ----
        tmpL = persist.tile([128, 2, 128], F32, tag="tmpL")
        resL = persist.tile([128, 2, 128], F32, tag="resL")
        outrL = persist.tile([128, 2, 2], F32, tag="outrL")
        for ri, (r0, nr) in enumerate(ROW_TILES):
            last_tile = ri == len(ROW_TILES) - 1
            outr = outrL if last_tile else small4.tile(
                [128, 2, 8], F32, tag="outr", name=f"outr{ri}")
            for oc in range(2):
                pc = psum1.tile([128, 512], F32, name=f"c1ps{ri}_{oc}",
                                tag="ps_acc" if oc == 0 else "ps_cv")
                pcv = pc[:, :nr * W].rearrange("p (r w) -> p r w", w=W)
                which, lr0 = rd(r0)
                chh = c0hT if which == "T" else c0hB
                cll = c0lT if which == "T" else c0lB
                terms1 = [(w1h_s, chh), (w1h_s, cll), (w1l_s, chh)]
                i_mm, n_mm = 0, 9 * len(terms1)
                for w_s, x_s in terms1:
                    for t9 in range(9):
                        dh, dw = divmod(t9, 3)
                        nc.tensor.matmul(
                            pcv,
                            w_s[:, t9 * 2:t9 * 2 + 2, oc * 128:(oc + 1) * 128],
                            x_s[:, :, lr0 + dh - 1:lr0 + dh - 1 + nr,
                                dw:dw + W],
                            start=(i_mm == 0), stop=(i_mm == n_mm - 1),
                            perf_mode=DR)
                        i_mm += 1
                tmp = (tmpL[:, oc, :] if last_tile else
                       small4.tile([128, 512], F32, tag="scratch",
                                   name=f"tmp{ri}_{oc}"))
                nc.scalar.activation(tmp[:, :nr * W], pc[:, :nr * W], AF.Gelu,
                                     bias=b1_s[:, oc:oc + 1],
                                     scale=a1_s[:, oc:oc + 1])
                yres, yr0 = yrd(r0, nr)
                res = (resL[:, oc, :] if last_tile else
                       small4.tile([128, 512], F32, tag="scratch",
                                   name=f"res{ri}_{oc}"))
                nc.vector.tensor_add(
                    res[:, :nr * W].rearrange("p (r w) -> p r w", w=W),
                    tmp[:, :nr * W].rearrange("p (r w) -> p r w", w=W),
                    yres[:, oc, yr0:yr0 + nr, 1:65])
                nc.vector.reduce_max(
                    outr[:, oc, :nr],
                    res[:, :nr * W].rearrange("p (r w) -> p r w", w=W),
                    axis=AX.X)
            # one DMA per row-tile (both oc halves); ACT for the last (its
            # queue is idle after the final gelu)
            eng = nc.scalar if last_tile else nc.sync
            eng.dma_start(out=outv[:, :, r0 - 1:r0 - 1 + nr],
                          in_=outr[:, :, :nr])

    nc.compile()
    return nc


def _prep_maps(inputs):
    """Host-side input prep: slicing, transposes, BN folding, fp8 splits."""
    f = np.float32
    in0 = np.ascontiguousarray(np.asarray(inputs["inputs_0"], f).reshape(B, C, N))
    in1 = np.ascontiguousarray(np.asarray(inputs["inputs_1"], f).reshape(B, C, N))
    g = {k: np.asarray(v, f) for k, v in inputs.items()}

    def fold(gm, bt, m, v, conv_b):
        a = (gm / np.sqrt(v + EPS)).astype(f)
        return a, (bt - m * a + a * conv_b).astype(f)

    a_bn, b_bn = fold(g["bn0_g"], g["bn0_b"], g["bn0_m"], g["bn0_v"],
                      g["b_o"] + g["w_o"] @ g["b_q"])
    a0, b0 = fold(g["cb_bn0_g"], g["cb_bn0_b"], g["cb_bn0_m"], g["cb_bn0_v"],
                  g["cb_b0"])
    a1, b1 = fold(g["cb_bn1_g"], g["cb_bn1_b"], g["cb_bn1_m"], g["cb_bn1_v"],
                  g["cb_b1"])

    def wsplit(w):
        wh = w.astype(F8NP)
        wl = (w - wh.astype(f)).astype(F8NP)
        return wh, wl

    def cc_pack(wt):
        # [C, Ch] -> [128, 2, 128] with the C dim split (a p)
        return wt.reshape(2, 128, Ch).transpose(1, 0, 2)

    def cc_pack_hl(w):
        # w [Ch, C] -> hi/lo fp8 [128, 2, 128] pair of w.T
        wh = w.T.astype(F8NP).astype(f)
        wl = (w.T - wh).astype(F8NP)
        return cc_pack(wh.astype(F8NP)), cc_pack(wl)

    def act_pack_hl(x3d):
        # x [128, 2, n] f32 -> [128, 2, 2n] fp8 [hi | lo]
        xh = x3d.astype(F8NP)
        xl = (x3d - xh.astype(f)).astype(F8NP)
        return np.concatenate([xh, xl], axis=2)

    # conv weights as (tap, ci, o); x0-half naive fp8, in0-half + w1 hi/lo
    w0t = np.ascontiguousarray(
        g["cb_w0"].transpose(2, 3, 1, 0).reshape(9, 2 * C, C))
    w1t = np.ascontiguousarray(
        g["cb_w1"].transpose(2, 3, 1, 0).reshape(9, C, C))
    w0inh, w0inl = wsplit(w0t[:, C:, :])
    w1h, w1l = wsplit(w1t)

    def conv_pack(stack):
        # [s, 9, C, C] -> [128, s*9*2, C] matching "s t (a p) o -> p (s t a) o"
        s = stack.shape[0]
        return np.ascontiguousarray(
            stack.reshape(s, 9, 2, 128, C).transpose(3, 0, 1, 2, 4)
            .reshape(128, s * 18, C))

    w01 = np.concatenate([
        conv_pack(np.stack([w0t[:, :C, :].astype(F8NP), w0inh, w0inl])),
        conv_pack(np.stack([w1h, w1l]))], axis=1)

    wqh, wql = cc_pack_hl(g["w_q"])
    wkh, wkl = cc_pack_hl(g["w_k"])
    wvh, wvl = cc_pack_hl(g["w_v"])
    wqk = np.ascontiguousarray(np.concatenate([wqh, wql, wkh, wkl], axis=1))
    wvhl = np.ascontiguousarray(np.concatenate([wvh, wvl], axis=1))
    wvoi = np.ascontiguousarray(np.concatenate(
        [g["w_o"].T, np.eye(128, dtype=f)], axis=1)).astype(BF16NP)
    bias6 = np.stack([a_bn, b_bn, a0, b0, a1, b1])  # [6, 256]
    biasb = np.ascontiguousarray(np.concatenate(
        [bias6.reshape(6, 2, 128).transpose(2, 0, 1).reshape(128, 12),
         g["b_v"].reshape(128, 1)], axis=1)).astype(f)

    shared = {
        "wqk": wqk,
        "wvhl": wvhl,
        "wvoi": wvoi,
        "biasb": biasb,
        "w01": np.ascontiguousarray(w01),
    }
    maps = []
    for b in range(B):
        # fp8 hi|lo packed per 512-col quarter: [128, 2cc, 8x(512h|512l)]
        i03 = in0[b].reshape(2, 128, N).transpose(1, 0, 2)
        q8 = [act_pack_hl(np.ascontiguousarray(i03[:, :, q * 512:(q + 1) * 512]))
              for q in range(8)]
        in0p = np.ascontiguousarray(np.concatenate(q8, axis=2))
        for half in range(2):
            w0r = 0 if half == 0 else 30
            sl = slice(w0r * W, (w0r + ROWS) * W)
            in0w_f32 = in0[b][:, sl].reshape(C, ROWS, W)
            in0h = np.zeros((C, ROWS, 66), F8NP)
            in0l = np.zeros((C, ROWS, 66), F8NP)
            in0h[:, :, 1:65] = in0w_f32.astype(F8NP)
            in0l[:, :, 1:65] = (
                in0w_f32 - in0h[:, :, 1:65].astype(f)).astype(F8NP)
            i13 = in1[b][:, sl].reshape(2, 128, KW).transpose(1, 0, 2)
            in1p = np.ascontiguousarray(np.concatenate(
                [act_pack_hl(np.ascontiguousarray(i13[:, :, a:b2]))
                 for a, b2 in [(0, 512), (512, 1024), (1024, KW)]], axis=2))
            maps.append({
                "in0p": in0p,
                "in0h": in0h.reshape(C, ROWS * 66),
                "in0l": in0l.reshape(C, ROWS * 66),
                "in1p": in1p,
                **shared,
            })
    return maps


def kernel(**inputs):
    if "nc" not in _CACHED:
        _CACHED["nc"] = build_program()
    nc = _CACHED["nc"]
    maps = _prep_maps(inputs)
    res = run_bass_kernel_spmd(nc, maps, core_ids=list(range(8)))
    out = np.zeros((B, C), np.float32)
    for b in range(B):
        top = res.results[2 * b]["out"][:, 0:32].max(axis=1)
        bot = res.results[2 * b + 1]["out"][:, 2:34].max(axis=1)
        out[b] = np.maximum(out[b], np.maximum(top, bot))
    return out


# revision 33
# speedup vs baseline: 1.0842x; 1.0842x over previous
"""Trainium2 Bass kernel for nn_CFAConv (cross-feature attention + conv block).

Self-contained: takes full unsharded inputs, shards (batch, image-half) across
8 NeuronCores, runs one SPMD Bass/Tile NEFF, and combines partial results on
the host.

Math (validated against the jax reference in numpy):
  x1 = w_q@in0 + b_q ; x2 = w_k@in0 + b_k ; x3 = w_v@in1 + b_v  (1x1 convs)
  aff = softmax_j(x2^T x3) ; x0 = x1 @ aff
  x0' = gelu(bn0(w_o@x0 + b_o))
  y = gelu(bn(conv3x3(concat(x0', in0)))) ; y = gelu(bn(conv3x3(y)))
  out = max_spatial(y + x0')
On-device simplifications:
  - softmax over j is invariant to per-column shifts => b_k drops entirely
  - x2^T(x3 + b_v) = x2^T x3 + (x2^T b_v) 1^T    => fold b_v into x3
  - (x1 + b_q 1^T) @ aff = x1@aff + b_q 1^T (aff columns sum to 1)
    => fold w_o@b_q into the out-projection bias (host-side)
  - eval-mode BN folds to per-channel scale/bias, fused into the gelu ACT op
  - softmax normalization deferred past the x1@exp(S) matmul (divide x0 by
    column sums); sums via a 5-level bf16 DVE pre-sum tree + one ones-matmul
  - no max-subtraction in softmax: |S| <= ~60 here; exp fits fp32 (max ~e88)
Precision: bf16 operands with fp32 PSUM accumulation for the attention path;
the two 3x3 convs run in fp8e4m3 with DoubleRow perf mode (2 contraction
tiles per pass at 0.5 cycles/row):
  - conv0 x0'-half: weights + acts naive fp8 (x0' is small vs in0 => cheap)
  - conv0 in0-half: weights hi+lo fp8 split, in0 hi+lo fp8 split (host-side),
    3-term product (Wh Xh + Wh Xl + Wl Xh)
  - conv1: weights hi+lo (host), c0 hi+lo split on DVE, 3-term
  (numpy bit-model: 1.3e-2 final rel err vs the 2e-2 budget)
Scheduling (v2): DMAs are packed (fewer issues; the shared HWDGE serializes
at ~630ns per DMA) and ordered so the projection inputs stream in consumption
order; x2/x1t/x3 live in per-chunk tiles so each S matmul gates only on its
own quarter; the attention is software-pipelined: S+exp of k-tile kt+1 are
interleaved into the x0 h-blocks of k-tile kt, the colsum tree runs per
h-block, and the out-projection is folded into the same loop. conv0's
in0-half partial sums (27 DoubleRow passes each) fill the PE slack of the
ACT-exp-bound attention phase.
Sharding: 8 cores = (4 batches) x (top/bottom image half). Each core computes
a 34-row window (32 owned + halo) so the two 3x3 convs need no communication;
per-row maxes [256, 34] go to the host which slices owned rows and reduces.
"""

from contextlib import ExitStack

import ml_dtypes
import numpy as np

import concourse.bass as bass
import concourse.tile as tile
from concourse import bacc, mybir
from concourse.bass_utils import run_bass_kernel_spmd

B, C, H, W = 4, 256, 64, 64
Ch = C // 2          # 128
N = H * W            # 4096
ROWS = 34            # per-core row window (32 owned + 2 halo)
KW = ROWS * W        # 2176 window positions
EPS = 1e-5

F32 = mybir.dt.float32
BF16 = mybir.dt.bfloat16
F8 = mybir.dt.float8e4
AF = mybir.ActivationFunctionType
AX = mybir.AxisListType
DR = mybir.MatmulPerfMode.DoubleRow
BF16NP = ml_dtypes.bfloat16
F8NP = ml_dtypes.float8_e4m3

# attention k-tiles over the 2176-column window
K_TILES = [(0, 512), (512, 512), (1024, 512), (1536, 512), (2048, 128)]
# conv output row-tiles (local rows 1..34 of the 36-row padded buffer)
ROW_TILES = [(1, 8), (9, 8), (17, 8), (25, 8), (33, 2)]

_CACHED = {}


def build_program():
    nc = bacc.Bacc("TRN2", target_bir_lowering=False, debug=False)

    def din(name, shape, dt=F32):
        return nc.dram_tensor(name, shape, dt, kind="ExternalInput").ap()

    # in0/in1 ship as fp8 hi|lo pairs packed per chunk (same bytes as bf16)
    in0p_d = din("in0p", [128, 2, 2 * N], F8)
    in1p_d = din("in1p", [128, 2, 2 * KW], F8)
    # in0 conv window, fp8 hi/lo, pre-padded to 66 cols (zero side columns)
    in0h_d = din("in0h", [C, ROWS * 66], F8)
    in0l_d = din("in0l", [C, ROWS * 66], F8)
    # packed small weights: one DMA each (HWDGE issue is ~630ns/DMA)
    wqk_d = din("wqk", [128, 8, 128], F8)      # [wq_h|wq_l|wk_h|wk_l] (2cc each)
    wvhl_d = din("wvhl", [128, 4, 128], F8)    # [wv_h|wv_l]
    wvoi_d = din("wvoi", [128, 384], BF16)     # wo(256) | id(128)
    biasb_d = din("biasb", [128, 13])          # bias6 (12) | bv (1)
    # conv weights: [x0-half naive, in0 hi, in0 lo, w1 hi, w1 lo] (tap, ci, o)
    w01_d = din("w01", [128, 90, C], F8)
    out = nc.dram_tensor("out", [C, ROWS], F32, kind="ExternalOutput").ap()
    outv = out.rearrange("(a p) r -> p a r", a=2)

    with tile.TileContext(nc) as tc, ExitStack() as ctx:
        persist = ctx.enter_context(tc.tile_pool(name="persist", bufs=1))
        psum = ctx.enter_context(tc.tile_pool(name="psum", bufs=2, space="PSUM"))
        psum1 = ctx.enter_context(tc.tile_pool(name="psum1", bufs=2, space="PSUM"))
        small = ctx.enter_context(tc.tile_pool(name="small", bufs=3))
        small4 = ctx.enter_context(tc.tile_pool(name="small4", bufs=4))

        # ---- input DMAs, in consumption order (transfers serialize at
        # ~358GB/s; HWDGE issues at ~630ns) ----
        # Queue strategy: no input DMAs on the scalar (ACT) queue — each
        # dispatch blocks the ACT sequencer ~1.26us, delaying the first exps.
        # Front-critical inputs go on the SP/HWDGE queue in consumption
        # order (in1a early: it gates S(0,0) and so the whole ACT exp chain).

        wqk_s = persist.tile([128, 8, 128], F8, tag="wqk")
        nc.sync.dma_start(out=wqk_s, in_=wqk_d)
        wqh_s, wql_s = wqk_s[:, 0:2, :], wqk_s[:, 2:4, :]
        wkh_s, wkl_s = wqk_s[:, 4:6, :], wqk_s[:, 6:8, :]
        in0q = [persist.tile([128, 2, 1024], F8, tag=f"in0q{q}",
                             name=f"in0q{q}") for q in range(8)]
        nc.sync.dma_start(out=in0q[0], in_=in0p_d[:, :, 0:1024])
        in1q = [persist.tile([128, 2, 1024], F8, tag="in1a1", name="in1a1"),
                persist.tile([128, 2, 1024], F8, tag="in1a2", name="in1a2"),
                persist.tile([128, 2, 2304], F8, tag="in1b", name="in1b")]
        nc.sync.dma_start(out=in1q[0], in_=in1p_d[:, :, 0:1024])
        wvhl_s = persist.tile([128, 4, 128], F8, tag="wvhl")
        nc.sync.dma_start(out=wvhl_s, in_=wvhl_d)
        wvh_s, wvl_s = wvhl_s[:, 0:2, :], wvhl_s[:, 2:4, :]
        wvoi_s = persist.tile([128, 384], BF16, tag="wvoi")
        nc.sync.dma_start(out=wvoi_s, in_=wvoi_d)
        wo_s = wvoi_s[:, 0:256]
        id_s = wvoi_s[:, 256:384]
        biasb_s = persist.tile([128, 13], F32, tag="biasb")
        nc.sync.dma_start(out=biasb_s, in_=biasb_d)
        ao_s, bo_s = biasb_s[:, 0:2], biasb_s[:, 2:4]
        a0_s, b0_s = biasb_s[:, 4:6], biasb_s[:, 6:8]
        a1_s, b1_s = biasb_s[:, 8:10], biasb_s[:, 10:12]
        bv_s = biasb_s[:, 12:13]
        nc.sync.dma_start(out=in0q[1], in_=in0p_d[:, :, 1024:2048])
        nc.sync.dma_start(out=in1q[1], in_=in1p_d[:, :, 1024:2048])
        for q in range(2, 8):
            nc.sync.dma_start(out=in0q[q],
                              in_=in0p_d[:, :, q * 1024:(q + 1) * 1024])

        # conv inputs/weights (needed from ~28us; back of the SP queue)
        convbuf = ctx.enter_context(tc.tile_pool(name="convbuf", bufs=1))
        in0h_s = convbuf.tile([128, 2, 36, 66], F8, tag="in0h")
        in0l_s = convbuf.tile([128, 2, 36, 66], F8, tag="in0l")
        nc.sync.dma_start(out=in1q[2], in_=in1p_d[:, :, 2048:2 * KW])
        nc.sync.dma_start(
            out=in0h_s[:, :, 1:35, :],
            in_=in0h_d.rearrange("(a p) n -> p a n", a=2))
        nc.sync.dma_start(
            out=in0l_s[:, :, 1:35, :],
            in_=in0l_d.rearrange("(a p) n -> p a n", a=2))
        w01_s = persist.tile([128, 90, C], F8, tag="w01")
        nc.sync.dma_start(out=w01_s, in_=w01_d)
        w0x_s = w01_s[:, 0:18]
        w0inh_s = w01_s[:, 18:36]
        w0inl_s = w01_s[:, 36:54]
        w1h_s = w01_s[:, 54:72]
        w1l_s = w01_s[:, 72:90]

        ones_s = persist.tile([128, 1], BF16, tag="ones")
        nc.vector.memset(ones_s, 1.0)
        ones_row = persist.tile([1, 128], BF16, tag="ones_row")
        nc.vector.memset(ones_row, 1.0)

        # p-state warmup: the PE ramps to full clock only after 3us of
        # continuous execution; spin it on throwaway matmuls while the first
        # input DMAs are in flight
        spin_src = persist.tile([128, 256], BF16, tag="spin_src")
        nc.vector.memset(spin_src, 1.0)
        spin_ps = psum1.tile([128, 512], F32, tag="ps_cv", name="spin_ps")
        for _ in range(5):
            nc.tensor.matmul(spin_ps[0:1, :256], ones_s, spin_src,
                             start=True, stop=True)
        spin_sink = small.tile([1, 16], F32, tag="spin_sink")
        nc.vector.tensor_copy(spin_sink, spin_ps[0:1, :16])  # ring consumer

        # ---- conv pad zeroing on the otherwise-idle Pool engine ----
        TOPR, BOTR0 = 19, 16  # top rows [0,19), bottom rows [16,36)
        # ybuf in THREE pieces so conv0's row tiles gate on the narrowest
        # set of out-projection writers (tile deps are whole-tile):
        # T rows [0,19), M rows [16,27), B rows [24,36)
        ybufT = convbuf.tile([128, 2, TOPR, 66], F8, tag="ybufT")
        ybufM = convbuf.tile([128, 2, 11, 66], F8, tag="ybufM")
        ybufB = convbuf.tile([128, 2, 12, 66], F8, tag="ybufB")
        YPIECES = [("T", ybufT, 0, 19), ("M", ybufM, 16, 27),
                   ("B", ybufB, 24, 36)]
        c0hT = convbuf.tile([128, 2, TOPR, 66], F8, tag="c0hT")
        c0hB = convbuf.tile([128, 2, 36 - BOTR0, 66], F8, tag="c0hB")
        c0lT = convbuf.tile([128, 2, TOPR, 66], F8, tag="c0lT")
        c0lB = convbuf.tile([128, 2, 36 - BOTR0, 66], F8, tag="c0lB")
        c0f = convbuf.tile([128, 2, ROWS, W], BF16, tag="c0f")
        def pad_memsets():
            # on DVE (mid-queue): Pool must stay free for the x3 bias-adds,
            # and the ACT/SP queues are on critical paths
            for tl in (in0h_s, in0l_s):
                nc.vector.memset(tl[:, :, 0, :], 0.0)
                nc.vector.memset(tl[:, :, 35, :], 0.0)
            for tl in (ybufT, c0hT, c0lT):
                nc.vector.memset(tl[:, :, 0, :], 0.0)
                nc.vector.memset(tl[:, :, 1:, 0:1], 0.0)
                nc.vector.memset(tl[:, :, 1:, 65:66], 0.0)
            nc.vector.memset(ybufM[:, :, :, 0:1], 0.0)
            nc.vector.memset(ybufM[:, :, :, 65:66], 0.0)
            nc.vector.memset(ybufB[:, :, 11, :], 0.0)
            nc.vector.memset(ybufB[:, :, :11, 0:1], 0.0)
            nc.vector.memset(ybufB[:, :, :11, 65:66], 0.0)
            for tl in (c0hB, c0lB):
                nc.vector.memset(tl[:, :, 36 - BOTR0 - 1, :], 0.0)
                nc.vector.memset(tl[:, :, :36 - BOTR0 - 1, 0:1], 0.0)
                nc.vector.memset(tl[:, :, :36 - BOTR0 - 1, 65:66], 0.0)

        # ---- projections into per-chunk tiles (deps gate per quarter) ----
        x2_t = [persist.tile([128, 512], BF16, tag=f"x2_{q}", name=f"x2_{q}")
                for q in range(8)]
        x1t_t = [persist.tile([128, 4, Ch], BF16, tag=f"x1t_{q}",
                              name=f"x1t_{q}") for q in range(8)]
        x3_t = [persist.tile([128, K_TILES[kt][1]], BF16, tag=f"x3_{kt}",
                             name=f"x3_{kt}") for kt in range(5)]

        def proj_x2(jc):
            # x2 chunk: 3-term fp8 DoubleRow (Wh Xh + Wl Xh + Wh Xl), one
            # accumulation group in one PSUM bank. On the ps_acc ring
            # (DVE-drained): the ps_S ring is reserved for x3 + S pairs
            # whose consumers (ACT exps) are slow.
            ps2 = psum1.tile([128, 512], F32, tag="ps_acc",
                             name=f"ps2_{jc}")
            t = in0q[jc]
            terms = [(wkh_s, 0), (wkl_s, 0), (wkh_s, 512)]
            for i, (w_s, xo) in enumerate(terms):
                nc.tensor.matmul(ps2, w_s, t[:, :, xo:xo + 512],
                                 start=(i == 0), stop=(i == 2),
                                 perf_mode=DR)
            nc.vector.tensor_copy(x2_t[jc], ps2)

        def proj_x1t(jc, tag="ps_acc"):
            # x1t naive fp8 (Wh Xh only): x1's error enters x0 linearly, not
            # through the exp; numpy bit-model 1.49e-2 vs the 2e-2 budget
            ps1 = psum1.tile([128, 512], F32, tag=tag, name=f"ps1_{jc}")
            t = in0q[jc]
            for js in range(4):
                nc.tensor.matmul(
                    ps1[:, js * 128:js * 128 + Ch],
                    t[:, :, js * 128:js * 128 + 128], wqh_s,
                    start=(js == 0), stop=(js == 3),
                    perf_mode=DR)
            nc.vector.tensor_copy(x1t_t[jc],
                                  ps1.rearrange("p (a c) -> p a c", c=Ch))

        def x3_tile(kt):
            k0, ksz = K_TILES[kt]
            iq, off, hw = ((kt, 0, 512) if k0 < 1024 else
                           (2, k0 - 1024, 1152))
            ps3 = psum.tile([128, 2, 512], F32, tag="ps_S", name=f"ps3_{kt}")
            t = in1q[iq]
            terms = [(wvh_s, 0), (wvl_s, 0), (wvh_s, hw)]
            for i, (w_s, xo) in enumerate(terms):
                nc.tensor.matmul(ps3[:, 0, :ksz], w_s,
                                 t[:, :, xo + off:xo + off + ksz],
                                 start=(i == 0), stop=(i == 2),
                                 perf_mode=DR)
            # x3 = psum + b_v : folds the v-bias into the affinity logits
            nc.vector.tensor_scalar_add(x3_t[kt][:, :ksz], ps3[:, 0, :ksz],
                                        bv_s)

        # ---- conv0 in0-half partials (fill PE slack in the attention) ----
        inpart = [persist.tile([128, 512], BF16, tag=f"inpart{g}",
                               name=f"inpart{g}") for g in range(10)]
        terms_in0 = [(w0inh_s, in0h_s), (w0inh_s, in0l_s), (w0inl_s, in0h_s)]
        part_ps = {}

        def partial_chunk(g, p0, p1):
            """DoubleRow passes [p0,p1) of the 27 for conv0's in0-half group
            g=(rt,oc); chunked so the passes can be spread between attention
            h-blocks as PE filler; evicted to SBUF bf16 on the last chunk."""
            (r0, nr), oc = ROW_TILES[g // 2], g % 2
            if p0 == 0:
                part_ps[g] = psum1.tile([128, 512], F32, tag="ps_cv",
                                        name=f"cv{g}")
            ps = part_ps[g]
            pcv = ps[:, :nr * W].rearrange("p (r w) -> p r w", w=W)
            for i_mm in range(p0, p1):
                w_s, x_s = terms_in0[i_mm // 9]
                dh, dw = divmod(i_mm % 9, 3)
                nc.tensor.matmul(
                    pcv,
                    w_s[:, (i_mm % 9) * 2:(i_mm % 9) * 2 + 2,
                        oc * 128:(oc + 1) * 128],
                    x_s[:, :, r0 + dh - 1:r0 + dh - 1 + nr, dw:dw + W],
                    start=(i_mm == 0), stop=(i_mm == 26),
                    perf_mode=DR)
            if p1 == 27:
                nc.vector.tensor_copy(inpart[g][:, :nr * W], ps[:, :nr * W])

        def emit_in0_partial(g):
            partial_chunk(g, 0, 27)

        # ---- software-pipelined attention ----
        attn = ctx.enter_context(tc.tile_pool(name="attn", bufs=4))
        attn2 = ctx.enter_context(tc.tile_pool(name="attn2", bufs=2))
        attnE = ctx.enter_context(tc.tile_pool(name="attnE", bufs=18))
        x0n_t = [persist.tile([128, 512], BF16, tag=f"x0n{kt}",
                              name=f"x0n{kt}") for kt in range(len(K_TILES))]
        expS_cur = {}

        def S_block(kt, mh):
            """S matmuls for chunk-pair mh of k-tile kt + the exp into a
            per-pair expS ring tile (x0 then gates on ONE exp, not four)."""
            k0, ksz = K_TILES[kt]
            et = attnE.tile([128, 2, 512], BF16, tag="expS",
                            name=f"expS{kt}_{mh}")
            expS_cur[(kt, mh)] = et
            sp = psum.tile([128, 2, 512], F32, tag="ps_S",
                           name=f"S{kt}_{mh}")
            for i in range(2):
                m = 2 * mh + i
                nc.tensor.matmul(
                    sp[:, i, :ksz],
                    x2_t[m // 4][:, (m % 4) * 128:(m % 4 + 1) * 128],
                    x3_t[kt][:, :ksz],
                    start=True, stop=True)
            nc.scalar.activation(et[:, :, :ksz], sp[:, :, :ksz], AF.Exp)

        part_x1 = {}

        def proj_x1t_chunk(jc, c0, c1):
            """passes [c0,c1) of quarter jc's 4-pass x1t group (deferred
            into the kt0 pair-loop as PE filler; psum on the ps_cv ring)"""
            if c0 == 0:
                part_x1[jc] = psum1.tile([128, 512], F32, tag="ps_cv",
                                         name=f"ps1_{jc}")
            ps1 = part_x1[jc]
            t = in0q[jc]
            for js in range(c0, c1):
                nc.tensor.matmul(
                    ps1[:, js * 128:js * 128 + Ch],
                    t[:, :, js * 128:js * 128 + 128], wqh_s,
                    start=(js == 0), stop=(js == 3),
                    perf_mode=DR)
            if c1 == 4:
                nc.vector.tensor_copy(x1t_t[jc],
                                      ps1.rearrange("p (a c) -> p a c", c=Ch))

        # ---- front: x2 streams behind the input DMAs; x1t for quarters 2-7
        # is deferred into the kt0 pair-loop as PE filler (the attention
        # inner loop is ACT-exp-bound); S(kt0) starts once q0/q1 + x3t0 up ----
        proj_x2(0)
        proj_x1t(0)
        x3_tile(0)
        proj_x2(1)
        proj_x1t(1)
        S_block(0, 0)
        S_block(0, 1)
        S_block(0, 2)
        S_block(0, 3)
        x3_tile(1)
        proj_x2(2)
        S_block(0, 4)
        S_block(0, 5)
        proj_x2(3)
        S_block(0, 6)
        S_block(0, 7)
        proj_x2(4)
        S_block(0, 8)
        S_block(0, 9)
        proj_x2(5)
        S_block(0, 10)
        S_block(0, 11)
        proj_x2(6)
        S_block(0, 12)
        S_block(0, 13)
        proj_x2(7)
        S_block(0, 14)
        S_block(0, 15)
        pad_memsets()

        for kt, (k0, ksz) in enumerate(K_TILES):
            octs = attn2.tile([128, 4, 512], BF16, tag="oct",
                              name=f"oct{kt}")
            x0p = psum1.tile([128, 512], F32, tag="ps_acc", name=f"x0p{kt}")
            pr_t, qr_t = {}, {}
            filled = 0
            for mh in range(16):
                if kt + 1 < len(K_TILES):
                    S_block(kt + 1, mh)
                # PE filler BEFORE the exp-gated x0 pair, so PE chews on it
                # while ACT finishes exp(kt, mh)
                if kt == 0:
                    if 1 <= mh <= 6:
                        q = 2 + (mh - 1)
                        proj_x1t_chunk(q, 0, 2)
                        proj_x1t_chunk(q, 2, 4)
                    elif 7 <= mh <= 9:
                        x3_tile(mh - 5)   # x3 tiles 2, 3, 4
                    elif mh >= 12:
                        # group g0 fills kt0's tail (w01 lands ~22us)
                        tgt = 27 * (mh - 11) // 4
                        partial_chunk(0, filled, tgt)
                        filled = tgt
                else:
                    # two conv0-in0 partial groups per kt (54 DR passes)
                    tgt = 54 * (mh + 1) // 16
                    while filled < tgt:
                        g = 2 * kt - 1 + filled // 27
                        p = filled % 27
                        e = min(27, p + tgt - filled)
                        partial_chunk(g, p, e)
                        filled += e - p
                et = expS_cur[(kt, mh)]
                for i in range(2):
                    m = 2 * mh + i
                    nc.tensor.matmul(x0p[:, :ksz], x1t_t[m // 4][:, m % 4, :],
                                     et[:, i, :ksz],
                                     start=(m == 0), stop=(m == 31))
                pr = attn.tile([128, 512], BF16, tag="pr",
                               name=f"pr{kt}_{mh}")
                pr_t[mh] = pr
                nc.vector.tensor_add(pr[:, :ksz], et[:, 0, :ksz],
                                     et[:, 1, :ksz])
                if mh % 2 == 1:
                    qr = attn.tile([128, 512], BF16, tag="qr",
                                   name=f"qr{kt}_{mh//2}")
                    qr_t[mh // 2] = qr
                    nc.vector.tensor_add(qr[:, :ksz], pr_t[mh - 1][:, :ksz],
                                         pr_t[mh][:, :ksz])
                if mh % 4 == 3:
                    nc.vector.tensor_add(octs[:, mh // 4, :ksz],
                                         qr_t[mh // 2 - 1][:, :ksz],
                                         qr_t[mh // 2][:, :ksz])
            hexs = attn2.tile([128, 2, 512], BF16, tag="hex", name=f"hex{kt}")
            for i in range(2):
                nc.vector.tensor_add(hexs[:, i, :ksz], octs[:, 2 * i, :ksz],
                                     octs[:, 2 * i + 1, :ksz])
            top = attn2.tile([128, 512], BF16, tag="top", name=f"top{kt}")
            nc.vector.tensor_add(top[:, :ksz], hexs[:, 0, :ksz],
                                 hexs[:, 1, :ksz])
            ssum_t = psum1.tile([128, 512], F32, tag="ps_cv", name=f"ssum{kt}")
            nc.tensor.matmul(ssum_t[0:1, :ksz], ones_s, top[:, :ksz],
                             start=True, stop=True)
            sinv = small.tile([1, 512], BF16, tag="sinv")
            with nc.allow_low_precision(
                    reason="bf16 1/colsum only scales the softmax "
                           "normalization; 1.2e-2 in the numpy bit model"):
                nc.vector.reciprocal(sinv[:, :ksz], ssum_t[0:1, :ksz])
            # broadcast 1/colsum to all partitions with a 1-row bf16 matmul
            bcast = psum1.tile([128, 512], F32, tag="ps_cv",
                               name=f"sinv_bcast{kt}")
            nc.tensor.matmul(bcast[:, :ksz], ones_row, sinv[:, :ksz],
                             start=True, stop=True)
            sinvb = small.tile([128, 512], F32, tag="sinvb")
            nc.vector.tensor_copy(sinvb[:, :ksz], bcast[:, :ksz])
            nc.vector.tensor_mul(x0n_t[kt][:, :ksz], x0p[:, :ksz],
                                 sinvb[:, :ksz])

        # ---- out-projection + bn0 + gelu -> x0' (fp8) into ybuf. Kept as a
        # separate loop so the ACT stream stays [exps..., gelus...] — mixing
        # them costs a 1283ns activation-table load per switch. ----
        def outproj(kt):
            k0, ksz = K_TILES[kt]
            nr = ksz // W
            r0 = 1 + kt * 8
            for oc in range(2):
                po = psum1.tile([128, 512], F32, name=f"po{kt}_{oc}",
                                tag="ps_acc" if oc == 0 else "ps_cv")
                nc.tensor.matmul(po[:, :ksz],
                                 wo_s[:, oc * 128:(oc + 1) * 128],
                                 x0n_t[kt][:, :ksz],
                                 start=True, stop=True)
                pv = po[:, :ksz].rearrange("p (r w) -> p r w", w=W)
                for _, tl, p0, p1 in YPIECES:
                    lo, hi = max(r0, p0), min(r0 + nr, p1)
                    if lo < hi:
                        nc.scalar.activation(
                            tl[:, oc, lo - p0:hi - p0, 1:65],
                            pv[:, lo - r0:hi - r0],
                            AF.Gelu, bias=bo_s[:, oc:oc + 1],
                            scale=ao_s[:, oc:oc + 1])

        def rd(r0):
            """conv read window [r0-1, r0-1+nr+2) maps to exactly one tile"""
            return ("T", r0) if r0 < 16 else ("B", r0 - BOTR0)

        def yrd(r0, nr):
            """ybuf piece whose range contains rows [r0-1, r0+nr+1)"""
            for _, tl, p0, p1 in YPIECES:
                if r0 - 1 >= p0 and r0 + nr + 1 <= p1:
                    return tl, r0 - p0
            raise AssertionError(r0)

        def split_rows2(r0, nrows):
            parts = []
            t_hi = min(r0 + nrows, TOPR)
            if r0 < TOPR:
                parts.append(("T", r0, t_hi - r0, 0))
            b_lo = max(r0, BOTR0)
            if r0 + nrows > BOTR0:
                parts.append(("B", b_lo - BOTR0, r0 + nrows - b_lo, b_lo - r0))
            return parts

        # ---- conv0: x0'-half naive fp8 DoubleRow on top of the re-injected
        # in0-half partial (identity matmul opens the accumulation).
        # Tiles 0/1 read only ybufT (complete before outproj kt4), so they
        # are emitted first to cover kt4's serial ssum->recip->bcast->x0n
        # chain; outproj(kt4) then unblocks the ybufB tiles. ----
        def conv0_tile(ri):
            r0, nr = ROW_TILES[ri]
            for oc in range(2):
                pc = psum1.tile([128, 512], F32, name=f"c0ps{ri}_{oc}",
                                tag="ps_acc" if oc == 0 else "ps_cv")
                pcv = pc[:, :nr * W].rearrange("p (r w) -> p r w", w=W)
                ysrc, lr0 = yrd(r0, nr)
                nc.tensor.matmul(pc[:, :nr * W], id_s,
                                 inpart[ri * 2 + oc][:, :nr * W],
                                 start=True, stop=False)
                for t9 in range(9):
                    dh, dw = divmod(t9, 3)
                    nc.tensor.matmul(
                        pcv,
                        w0x_s[:, t9 * 2:t9 * 2 + 2, oc * 128:(oc + 1) * 128],
                        ysrc[:, :, lr0 + dh - 1:lr0 + dh - 1 + nr, dw:dw + W],
                        start=False, stop=(t9 == 8),
                        perf_mode=DR)
                nc.scalar.activation(
                    c0f[:, oc, r0 - 1:r0 - 1 + nr, :], pcv,
                    AF.Gelu, bias=b0_s[:, oc:oc + 1], scale=a0_s[:, oc:oc + 1])
                # hi/lo split of c0 for conv1's 3-term product (DVE), into
                # the top/bottom tiles (boundary rows land in both)
                for w2, lr2, n2, src2 in split_rows2(r0, nr):
                    chh = c0hT if w2 == "T" else c0hB
                    cll = c0lT if w2 == "T" else c0lB
                    s2 = r0 - 1 + src2
                    nc.vector.tensor_copy(chh[:, oc, lr2:lr2 + n2, 1:65],
                                          c0f[:, oc, s2:s2 + n2, :])
                    nc.vector.tensor_sub(cll[:, oc, lr2:lr2 + n2, 1:65],
                                         c0f[:, oc, s2:s2 + n2, :],
                                         chh[:, oc, lr2:lr2 + n2, 1:65])

        for kt in range(4):
            outproj(kt)
            if kt == 1:
                emit_in0_partial(9)
        conv0_tile(0)
        conv0_tile(1)
        conv0_tile(2)
        outproj(4)
        conv0_tile(3)
        conv0_tile(4)

        # ---- conv1: 256 -> 256, 3-term DoubleRow fp8, bn + gelu,
        #      + x0' residual, row-max; one output DMA per row-tile ----
        tmpL = persist.tile([128, 2, 128], F32, tag="tmpL")
        resL = persist.tile([128, 2, 128], F32, tag="resL")
        outrL = persist.tile([128, 2, 2], F32, tag="outrL")
        for ri, (r0, nr) in enumerate(ROW_TILES):
            last_tile = ri == len(ROW_TILES) - 1
            outr = outrL if last_tile else small4.tile(
                [128, 2, 8], F32, tag="outr", name=f"outr{ri}")
            for oc in range(2):
                pc = psum1.tile([128, 512], F32, name=f"c1ps{ri}_{oc}",
                                tag="ps_acc" if oc == 0 else "ps_cv")
                pcv = pc[:, :nr * W].rearrange("p (r w) -> p r w", w=W)
                which, lr0 = rd(r0)
                chh = c0hT if which == "T" else c0hB
                cll = c0lT if which == "T" else c0lB
                terms1 = [(w1h_s, chh), (w1h_s, cll), (w1l_s, chh)]
                i_mm, n_mm = 0, 9 * len(terms1)
                for w_s, x_s in terms1:
                    for t9 in range(9):
                        dh, dw = divmod(t9, 3)
                        nc.tensor.matmul(
                            pcv,
                            w_s[:, t9 * 2:t9 * 2 + 2, oc * 128:(oc + 1) * 128],
                            x_s[:, :, lr0 + dh - 1:lr0 + dh - 1 + nr,
                                dw:dw + W],
                            start=(i_mm == 0), stop=(i_mm == n_mm - 1),
                            perf_mode=DR)
                        i_mm += 1
                tmp = (tmpL[:, oc, :] if last_tile else
                       small4.tile([128, 512], F32, tag="scratch",
                                   name=f"tmp{ri}_{oc}"))
                nc.scalar.activation(tmp[:, :nr * W], pc[:, :nr * W], AF.Gelu,
                                     bias=b1_s[:, oc:oc + 1],
                                     scale=a1_s[:, oc:oc + 1])
                yres, yr0 = yrd(r0, nr)
                res = (resL[:, oc, :] if last_tile else
                       small4.tile([128, 512], F32, tag="scratch",
                                   name=f"res{ri}_{oc}"))
                nc.vector.tensor_add(
                    res[:, :nr * W].rearrange("p (r w) -> p r w", w=W),
                    tmp[:, :nr * W].rearrange("p (r w) -> p r w", w=W),
                    yres[:, oc, yr0:yr0 + nr, 1:65])
                nc.vector.reduce_max(
                    outr[:, oc, :nr],
                    res[:, :nr * W].rearrange("p (r w) -> p r w", w=W),
                    axis=AX.X)
            # one DMA per row-tile (both oc halves); ACT for the last (its
            # queue is idle after the final gelu)
            eng = nc.scalar if last_tile else nc.sync
            eng.dma_start(out=outv[:, :, r0 - 1:r0 - 1 + nr],
                          in_=outr[:, :, :nr])

    nc.compile()
    return nc


def _prep_maps(inputs):
    """Host-side input prep: slicing, transposes, BN folding, fp8 splits."""
    f = np.float32
    in0 = np.ascontiguousarray(np.asarray(inputs["inputs_0"], f).reshape(B, C, N))
    in1 = np.ascontiguousarray(np.asarray(inputs["inputs_1"], f).reshape(B, C, N))
    g = {k: np.asarray(v, f) for k, v in inputs.items()}

    def fold(gm, bt, m, v, conv_b):
        a = (gm / np.sqrt(v + EPS)).astype(f)
        return a, (bt - m * a + a * conv_b).astype(f)

    a_bn, b_bn = fold(g["bn0_g"], g["bn0_b"], g["bn0_m"], g["bn0_v"],
                      g["b_o"] + g["w_o"] @ g["b_q"])
    a0, b0 = fold(g["cb_bn0_g"], g["cb_bn0_b"], g["cb_bn0_m"], g["cb_bn0_v"],
                  g["cb_b0"])
    a1, b1 = fold(g["cb_bn1_g"], g["cb_bn1_b"], g["cb_bn1_m"], g["cb_bn1_v"],
                  g["cb_b1"])

    def wsplit(w):
        wh = w.astype(F8NP)
        wl = (w - wh.astype(f)).astype(F8NP)
        return wh, wl

    def cc_pack(wt):
        # [C, Ch] -> [128, 2, 128] with the C dim split (a p)
        return wt.reshape(2, 128, Ch).transpose(1, 0, 2)

    def cc_pack_hl(w):
        # w [Ch, C] -> hi/lo fp8 [128, 2, 128] pair of w.T
        wh = w.T.astype(F8NP).astype(f)
        wl = (w.T - wh).astype(F8NP)
        return cc_pack(wh.astype(F8NP)), cc_pack(wl)

    def act_pack_hl(x3d):
        # x [128, 2, n] f32 -> [128, 2, 2n] fp8 [hi | lo]
        xh = x3d.astype(F8NP)
        xl = (x3d - xh.astype(f)).astype(F8NP)
        return np.concatenate([xh, xl], axis=2)

    # conv weights as (tap, ci, o); x0-half naive fp8, in0-half + w1 hi/lo
    w0t = np.ascontiguousarray(
        g["cb_w0"].transpose(2, 3, 1, 0).reshape(9, 2 * C, C))
    w1t = np.ascontiguousarray(
        g["cb_w1"].transpose(2, 3, 1, 0).reshape(9, C, C))
    w0inh, w0inl = wsplit(w0t[:, C:, :])
    w1h, w1l = wsplit(w1t)

    def conv_pack(stack):
        # [s, 9, C, C] -> [128, s*9*2, C] matching "s t (a p) o -> p (s t a) o"
        s = stack.shape[0]
        return np.ascontiguousarray(
            stack.reshape(s, 9, 2, 128, C).transpose(3, 0, 1, 2, 4)
            .reshape(128, s * 18, C))

    w01 = np.concatenate([
        conv_pack(np.stack([w0t[:, :C, :].astype(F8NP), w0inh, w0inl])),
        conv_pack(np.stack([w1h, w1l]))], axis=1)

    wqh, wql = cc_pack_hl(g["w_q"])
    wkh, wkl = cc_pack_hl(g["w_k"])
    wvh, wvl = cc_pack_hl(g["w_v"])
    wqk = np.ascontiguousarray(np.concatenate([wqh, wql, wkh, wkl], axis=1))
    wvhl = np.ascontiguousarray(np.concatenate([wvh, wvl], axis=1))
    wvoi = np.ascontiguousarray(np.concatenate(
        [g["w_o"].T, np.eye(128, dtype=f)], axis=1)).astype(BF16NP)
    bias6 = np.stack([a_bn, b_bn, a0, b0, a1, b1])  # [6, 256]
    biasb = np.ascontiguousarray(np.concatenate(
        [bias6.reshape(6, 2, 128).transpose(2, 0, 1).reshape(128, 12),
         g["b_v"].reshape(128, 1)], axis=1)).astype(f)

    shared = {
        "wqk": wqk,
        "wvhl": wvhl,
        "wvoi": wvoi,
        "biasb": biasb,
        "w01": np.ascontiguousarray(w01),
    }
    maps = []
    for b in range(B):
        # fp8 hi|lo packed per 512-col quarter: [128, 2cc, 8x(512h|512l)]
        i03 = in0[b].reshape(2, 128, N).transpose(1, 0, 2)
        q8 = [act_pack_hl(np.ascontiguousarray(i03[:, :, q * 512:(q + 1) * 512]))
              for q in range(8)]
        in0p = np.ascontiguousarray(np.concatenate(q8, axis=2))
        for half in range(2):
            w0r = 0 if half == 0 else 30
            sl = slice(w0r * W, (w0r + ROWS) * W)
            in0w_f32 = in0[b][:, sl].reshape(C, ROWS, W)
            in0h = np.zeros((C, ROWS, 66), F8NP)
            in0l = np.zeros((C, ROWS, 66), F8NP)
            in0h[:, :, 1:65] = in0w_f32.astype(F8NP)
            in0l[:, :, 1:65] = (
                in0w_f32 - in0h[:, :, 1:65].astype(f)).astype(F8NP)
            i13 = in1[b][:, sl].reshape(2, 128, KW).transpose(1, 0, 2)
            in1p = np.ascontiguousarray(np.concatenate(
                [act_pack_hl(np.ascontiguousarray(i13[:, :, a:b2]))
                 for a, b2 in [(0, 512), (512, 1024), (1024, KW)]], axis=2))
            maps.append({
                "in0p": in0p,
                "in0h": in0h.reshape(C, ROWS * 66),
                "in0l": in0l.reshape(C, ROWS * 66),
                "in1p": in1p,
                **shared,
            })
    return maps


def kernel(**inputs):
    if "nc" not in _CACHED:
        _CACHED["nc"] = build_program()
    nc = _CACHED["nc"]
    maps = _prep_maps(inputs)
    res = run_bass_kernel_spmd(nc, maps, core_ids=list(range(8)))
    out = np.zeros((B, C), np.float32)
    for b in range(B):
        top = res.results[2 * b]["out"][:, 0:32].max(axis=1)
        bot = res.results[2 * b + 1]["out"][:, 2:34].max(axis=1)
        out[b] = np.maximum(out[b], np.maximum(top, bot))
    return out
